# revision 44
# baseline (speedup 1.0000x reference)
"""BiDAF on 8 trn2 cores. Data-parallel over batch (4/core), both LSTM dirs per core.

Layout conventions (per core, B_local=4):
  tok = t*4 + b  (t-major) within each stream (q: 64 steps, c: 512 steps)
  Activations transposed: [feat(128-chunks) partitions, tok free]
  2H feat-chunk order: c = hc*2 + dir  (hc = h-dim chunk 0/1, dir 0=fwd 1=bwd)
  Gate order permuted to (i, f, o, g); gate n-chunks nc 0..7 (i:0-1 f:2-3 o:4-5 g:6-7)
  Recurrence gates PSUM tile [128, 64]: free = nc*8 + dir*4 + b
  h/c state + hseq slots: [128, 16]: free = hc*8 + dir*4 + b
  hseq SBUF buffer per layer: [128, T*16], slot t at free [t*16, (t+1)*16)
  xprojT DRAM per layer: [(mi*2+dir)*128 + p, ntok] bf16, includes bias

Recurrence step (the wall-clock driver: 4x512 + 64 serial cells):
  - g-gates pre-scaled x2 in the packed weights so ONE sigmoid pass covers
    [i, f, o, 2g]; tanh(g) recovered on DVE as 2*sigmoid(2g)-1 (f32 to avoid
    bf16 cancellation near 0.5).
  - xproj folded into the gates PSUM via identity matmuls issued before the
    h-dependent Whh matmuls (single start=True per step; TRN2 zero-region
    first-touch semantics make later writes land on zeroed cells).
  - c update: u = 2A_g-1; P = [i,f]*[u,c]; c = P_lo + P_hi (3 DVE ops).
  - h written once to a ping-pong h_cur tile feeding next step's matmuls;
    hseq slot copies happen on the (otherwise idle) GPSIMD engine.
Inprojs: weights repacked host-side into per-(dir, mi) contiguous blocks
(one DMA each); ctx weights resident in SBUF; mod1/mod2/dec streamed; GT
loaded in two [128, 8x512] batched DMAs per t-chunk; bias applied via the
ACT/DVE per-partition bias operand instead of ones-row matmuls.
"""
import numpy as np
import sys, os

sys.path.insert(0, "/opt/trn_rl_repo")

import ml_dtypes

BF16 = ml_dtypes.bfloat16
V, E, H = 50000, 300, 256
B, T, J = 32, 512, 64
BL = 4          # batch per core
NC_ = 8         # cores
W_WIN = 32      # recurrence xproj window (steps)

_PROGRAM_CACHE = {}


def _gate_perm():
    # (i,f,g,o) -> (i,f,o,g)
    return np.r_[0:512, 768:1024, 512:768]


PERM512 = np.r_[0:128, 256:384, 128:256, 384:512]


def _pack_whh(whh, bihsum=None):
    """whh [2, 1024, 256] -> [2, 128, 2048] bf16 pack for lhsT tiles.

    g-gate rows (perm chunks 6-7) are pre-scaled by 2 so that
    tanh(x) = 2*sigmoid(2x) - 1 works off a single sigmoid pass."""
    gp = _gate_perm()
    out = np.zeros((2, 128, 2048), dtype=BF16)
    for d in range(2):
        wT = whh[d][gp, :].T.astype(np.float32)  # [256, 1024] rows=h-dims cols=perm gates
        wT = wT.copy()
        wT[:, 768:1024] *= 2.0
        for hc in range(2):
            for nc in range(8):
                out[d, :, (hc * 8 + nc) * 128:(hc * 8 + nc) * 128 + 128] = \
                    wT[hc * 128:(hc + 1) * 128, nc * 128:(nc + 1) * 128].astype(BF16)
    return out


def _pack_wih(wih, bih, bhh, in_perm=None, pad_to=None):
    """wih [2, 1024, D] -> wihT' [2, pad, 1024] bf16 with bias row at D.

    g-gate columns (perm 768:1024) pre-scaled by 2 (see _pack_whh)."""
    gp = _gate_perm()
    D = wih.shape[2]
    pad = pad_to if pad_to else D + 1
    out = np.zeros((2, pad, 1024), dtype=BF16)
    for d in range(2):
        w = wih[d][gp, :].astype(np.float32).copy()  # [1024, D]
        b = (bih[d] + bhh[d])[gp].astype(np.float32).copy()
        w[768:1024, :] *= 2.0
        b[768:1024] *= 2.0
        if in_perm is not None:
            w = w[:, in_perm]
        out[d, :D, :] = w.T.astype(BF16)
        out[d, D, :] = b.astype(BF16)
    return out


def _build_host_inputs(inputs, core):
    """Prepare per-core device input dict (numpy)."""
    f32 = np.float32
    q = np.asarray(inputs["question"])[core * BL:(core + 1) * BL]  # [4, 64]
    c = np.asarray(inputs["context"])[core * BL:(core + 1) * BL]   # [4, 512]
    emb = np.asarray(inputs["emb"], dtype=f32)

    # token streams, tok = t*4 + b
    q_ids = q.T.reshape(-1)   # [64*4]
    c_ids = c.T.reshape(-1)   # [512*4]
    ids = np.concatenate([q_ids, c_ids])            # [2304]
    x = emb[ids]                                    # [2304, 300]
    xT = np.zeros((384, 2304), dtype=BF16)
    xT[:300] = x.T.astype(BF16)
    dev = {"xembT": xT.reshape(3, 128, 2304)}

    # highway weights, block layout: [L, wch, p, kc, m] (m = output feature)
    # biases as per-partition columns: hw_bcols[p, (L*2+wch)*3 + mi] = b[mi*128+p]
    hwp = np.zeros((2, 2, 128, 3, 300), dtype=BF16)
    hwb = np.zeros((128, 12), dtype=f32)
    for L in range(2):
        for wch, (wk, bk) in enumerate((("hw_lin_w", "hw_lin_b"),
                                        ("hw_gate_w", "hw_gate_b"))):
            wT = np.zeros((384, 300), dtype=f32)
            wT[:300] = np.asarray(inputs[wk], f32)[L].T
            hwp[L, wch] = wT.reshape(3, 128, 300).transpose(1, 0, 2).astype(BF16)
            bpad = np.zeros(384, dtype=f32)
            bpad[:300] = np.asarray(inputs[bk], f32)[L]
            hwb[:, (L * 2 + wch) * 3:(L * 2 + wch) * 3 + 3] = bpad.reshape(3, 128).T
    dev["hw_wpk"] = hwp
    dev["hw_bcols"] = hwb

    g_perm = np.concatenate([PERM512 + 512 * i for i in range(4)])
    ctx_w = _pack_wih(np.asarray(inputs["ctx_wih"], f32),
                      np.asarray(inputs["ctx_bih"], f32),
                      np.asarray(inputs["ctx_bhh"], f32), None, 384)
    # wih biases as columns: bias_cols[p, (li*2+d)*8 + mi] = bias[li,d][mi*128+p]
    bias_cols = np.zeros((128, 64), dtype=f32)
    # ctx bias lives at row 300 (inside chunk 2); extract it and zero the row
    bias_cols[:, 0:16] = ctx_w[:, 300, :].astype(f32).reshape(
        2, 8, 128).transpose(2, 0, 1).reshape(128, 16)
    ctx_w[:, 300, :] = 0
    # resident layout [d, p, mi, kc, m] (contiguous per (d, p))
    dev["ctx_wpk"] = ctx_w.reshape(2, 3, 128, 8, 128).transpose(0, 2, 3, 1, 4).copy()
    for li, (nm, perm, kpad) in enumerate((("mod1", g_perm, 2049),
                                           ("mod2", PERM512, 513),
                                           ("dec", PERM512, 513))):
        w = _pack_wih(np.asarray(inputs[nm + "_wih"], f32),
                      np.asarray(inputs[nm + "_bih"], f32),
                      np.asarray(inputs[nm + "_bhh"], f32), perm, kpad)
        bias_cols[:, 16 * (li + 1):16 * (li + 2)] = w[:, kpad - 1, :].astype(
            f32).reshape(2, 8, 128).transpose(2, 0, 1).reshape(128, 16)
        nkc = (kpad - 1) // 128
        blk = w[:, :kpad - 1].reshape(2, nkc, 128, 8, 128)
        # streamed layout [d, mi, p, kc, m] (contiguous per (d, mi, p))
        dev[nm + "_wpk"] = blk.transpose(0, 3, 2, 1, 4).copy()
    dev["bias_cols"] = bias_cols

    whh = np.stack([_pack_whh(np.asarray(inputs[k + "_whh"], f32))
                    for k in ("ctx", "mod1", "mod2", "dec")])  # [4, 2, 128, 2048]
    dev["whh_pack"] = whh.astype(BF16)
    dev["ident"] = np.eye(128, dtype=BF16)

    aw = np.asarray(inputs["att_w"], f32)  # [1536]
    w1, w2, w3 = aw[:512][PERM512], aw[512:1024][PERM512], aw[1024:][PERM512]
    dev["att_w1"] = w1.reshape(4, 128).T.astype(BF16).copy()
    dev["att_w2"] = w2.reshape(4, 128).T.astype(BF16).copy()
    dev["att_w3"] = w3.reshape(4, 128).T.astype(f32).copy()  # [128, 4] chunk-major
    dev["att_b"] = np.asarray(inputs["att_b"], f32).reshape(1, 1)

    for nm in ("p1", "p2"):
        pw = np.asarray(inputs[nm + "_w"], f32)  # [2560]
        gpart = np.concatenate([pw[512 * i:512 * (i + 1)][PERM512] for i in range(4)])
        mpart = pw[2048:][PERM512]
        dev[nm + "G"] = gpart.reshape(16, 128).T.astype(BF16).copy()
        dev[nm + "M"] = mpart.reshape(4, 128).T.astype(BF16).copy()
        dev[nm + "b"] = np.asarray(inputs[nm + "_b"], f32).reshape(1, 1).astype(BF16)
    return dev


def build_program():
    import os as _os
    KPH = int(_os.environ.get("KPH", "9"))
    import concourse.bass as bass
    import concourse.mybir as mybir
    from concourse.tile import TileContext
    import concourse.tile_utils as tile_utils
    tile_utils.max_sbuf_usage = 208 * 1024

    dt = mybir.dt
    ALU = mybir.AluOpType
    AF = mybir.ActivationFunctionType
    AX = mybir.AxisListType

    nc = bass.Bass()
    f32, bf = dt.float32, dt.bfloat16

    # ---- I/O ----
    xembT = nc.dram_tensor("xembT", [3, 128, 2304], bf, kind="ExternalInput")
    hw_wpk = nc.dram_tensor("hw_wpk", [2, 2, 128, 3, 300], bf, kind="ExternalInput")
    hw_bcols = nc.dram_tensor("hw_bcols", [128, 12], f32, kind="ExternalInput")
    ctx_wpk = nc.dram_tensor("ctx_wpk", [2, 128, 8, 3, 128], bf, kind="ExternalInput")
    mod1_wpk = nc.dram_tensor("mod1_wpk", [2, 8, 128, 16, 128], bf, kind="ExternalInput")
    mod2_wpk = nc.dram_tensor("mod2_wpk", [2, 8, 128, 4, 128], bf, kind="ExternalInput")
    dec_wpk = nc.dram_tensor("dec_wpk", [2, 8, 128, 4, 128], bf, kind="ExternalInput")
    bias_cols_d = nc.dram_tensor("bias_cols", [128, 64], f32, kind="ExternalInput")
    whh_pack = nc.dram_tensor("whh_pack", [4, 2, 128, 2048], bf, kind="ExternalInput")
    ident_d = nc.dram_tensor("ident", [128, 128], bf, kind="ExternalInput")
    att_w1 = nc.dram_tensor("att_w1", [128, 4], bf, kind="ExternalInput")
    att_w2 = nc.dram_tensor("att_w2", [128, 4], bf, kind="ExternalInput")
    att_w3 = nc.dram_tensor("att_w3", [128, 4], f32, kind="ExternalInput")
    att_b = nc.dram_tensor("att_b", [1, 1], f32, kind="ExternalInput")
    p1G = nc.dram_tensor("p1G", [128, 16], bf, kind="ExternalInput")
    p1M = nc.dram_tensor("p1M", [128, 4], bf, kind="ExternalInput")
    p1b = nc.dram_tensor("p1b", [1, 1], bf, kind="ExternalInput")
    p2G = nc.dram_tensor("p2G", [128, 16], bf, kind="ExternalInput")
    p2M = nc.dram_tensor("p2M", [128, 4], bf, kind="ExternalInput")
    p2b = nc.dram_tensor("p2b", [1, 1], bf, kind="ExternalInput")
    out_d = nc.dram_tensor("out", [2, 2048], f32, kind="ExternalOutput")

    NQ, NCtok = 256, 2048  # q/c stream token counts

    with TileContext(nc) as tc:
        import contextlib
        est = contextlib.ExitStack()
        with est:
            dram = est.enter_context(tc.tile_pool(name="dram", bufs=1, space="DRAM"))
            const = est.enter_context(tc.tile_pool(name="const", bufs=1))
            persist = est.enter_context(tc.tile_pool(name="persist", bufs=1))
            wpool = est.enter_context(tc.tile_pool(name="wpool", bufs=1))
            rpool = est.enter_context(tc.tile_pool(name="rhs", bufs=2))
            spool = est.enter_context(tc.tile_pool(name="scratch", bufs=3))
            xpool = est.enter_context(tc.tile_pool(name="xpool", bufs=1))
            mpool = est.enter_context(tc.tile_pool(name="mpool", bufs=2))
            psum = est.enter_context(tc.tile_pool(name="psum", bufs=2, space="PSUM"))
            psg = psum

            # DRAM scratch
            xprojq_d = dram.tile([16 * 128, NQ], bf)
            xprojc_d = [dram.tile([16 * 128, NCtok], bf, tag=f"xp{i}", name=f"xp{i}") for i in range(4)]
            GT_d = dram.tile([16 * 128, NCtok], bf)

            # constants
            ident = const.tile([128, 128], bf)
            nc.sync.dma_start(ident[:], ident_d[:])
            ones_row = const.tile([1, 512], bf)
            nc.vector.memset(ones_row[:], 1.0)
            ones_col = const.tile([128, 1], bf)
            nc.vector.memset(ones_col[:], 1.0)
            w3_sb = const.tile([128, 4], f32)
            nc.sync.dma_start(w3_sb[:], att_w3[:])
            attb_sb = const.tile([1, 1], f32)
            nc.sync.dma_start(attb_sb[:], att_b[:])
            pvec = {}
            for nm, dr, sh in (("p1G", p1G, [128, 16]), ("p1M", p1M, [128, 4]),
                               ("p2G", p2G, [128, 16]), ("p2M", p2M, [128, 4]),
                               ("w1", att_w1, [128, 4]), ("w2", att_w2, [128, 4]),
                               ("p1b", p1b, [1, 1]), ("p2b", p2b, [1, 1])):
                tl = const.tile(sh, bf, tag=nm, name=nm)
                nc.sync.dma_start(tl[:], dr[:])
                pvec[nm] = tl

            # persistent state
            hseq_q = persist.tile([128, J * 16], bf, tag="hq")
            hseq_c = persist.tile([128, T * 16], bf, tag="hc")
            hseq_m1 = persist.tile([128, T * 16], bf, tag="hm1")
            hseq_m2 = persist.tile([128, T * 16], bf, tag="hm2")
            hseq_dc = persist.tile([128, T * 16], bf, tag="hdc")
            h_init = persist.tile([128, 16], bf, tag="hi")
            nc.vector.memset(h_init[:], 0.0)
            # X tile: [:, 0:16] = tanh(g) scratch (u), [:, 16:32] = c state
            x_uc = persist.tile([128, 32], f32, tag="xuc")
            h_cur = [persist.tile([128, 16], bf, tag=f"hcur{j}", name=f"hcur{j}")
                     for j in range(2)]
            whh_sb = [persist.tile([128, 2048], bf, tag=f"whh{d}", name=f"whh{d}") for d in range(2)]

            def hview(hs):
                return hs.rearrange("p (t hc d b) -> p t hc d b", hc=2, d=2, b=4)

            # ---------------- resident weights (single batched DMA each) ----
            hw_sb = {}   # (L, wch) -> [128, 3, 300]
            hwb_all = wpool.tile([128, 12], f32, tag="hwb")
            nc.sync.dma_start(hwb_all[:], hw_bcols[:])
            for L in range(2):
                for wch in range(2):
                    t = wpool.tile([128, 3 * 300], bf, tag=f"hw{L}{wch}")
                    nc.sync.dma_start(
                        t.rearrange("p (kc m) -> p kc m", kc=3), hw_wpk[L, wch])
                    hw_sb[(L, wch)] = t.rearrange("p (kc m) -> p kc m", kc=3)
            # all wih biases as per-partition columns [128, (li*2+d)*8+mi]
            bias_all = wpool.tile([128, 64], f32, tag="biasall")
            nc.sync.dma_start(bias_all[:], bias_cols_d[:])
            LI = {"ctx": 0, "mod1": 1, "mod2": 2, "dec": 3}

            def bias_ap(nm, d, mi):
                col = (LI[nm] * 2 + d) * 8 + mi
                return bias_all[:, col:col + 1]

            # ctx weights resident; mod1/mod2/dec streamed per (d, mi)
            wres = {}
            for d in range(2):
                t = wpool.tile([128, 8 * 3 * 128], bf, tag=f"wctx{d}")
                nc.sync.dma_start(
                    t.rearrange("p (mi kc m) -> p mi kc m", mi=8, kc=3),
                    ctx_wpk[d])
                wres[("ctx", d)] = t.rearrange("p (mi kc m) -> p mi kc m",
                                               mi=8, kc=3)

            def stream_wt(wpk_dram, nkc_, nbuf=3):
                def get(d, mi, tk):
                    t = wpool.tile([128, nkc_ * 128], bf, tag=f"str{nkc_}",
                                   bufs=nbuf)
                    nc.sync.dma_start(
                        t.rearrange("p (kc m) -> p kc m", kc=nkc_),
                        wpk_dram[d, mi])
                    return t.rearrange("p (kc m) -> p kc m", kc=nkc_)
                return get

            # ---------------- highway ----------------
            xt = [xpool.tile([128, 2304], bf, tag=f"xt{c}", name=f"xt{c}") for c in range(3)]
            for c in range(3):
                nc.sync.dma_start(xt[c][:], xembT[c])
            mcs300 = [(0, 128), (128, 128), (256, 44)]
            for L in range(2):
                xo = [xpool.tile([128, 2304], bf, tag=(f"xt{c}" if L == 1 else f"xo{c}"), name=f"xo{L}{c}") for c in range(3)]
                nc.vector.memset(xo[2][:], 0.0)

                def hw_epi(ps_h, ps_t, mi, m0, msz, t0, tsz):
                    hh = mpool.tile([128, 512], bf, tag="hwh")
                    tt = mpool.tile([128, 512], bf, tag="hwt")
                    bcol_h = hwb_all[:msz, (L * 2) * 3 + mi:(L * 2) * 3 + mi + 1]
                    bcol_t = hwb_all[:msz, (L * 2 + 1) * 3 + mi:(L * 2 + 1) * 3 + mi + 1]
                    nc.scalar.activation(hh[:msz, :tsz], ps_h[:msz, :tsz], AF.Relu,
                                         bias=bcol_h)
                    nc.scalar.activation(tt[:msz, :tsz], ps_t[:msz, :tsz], AF.Relu,
                                         bias=bcol_t)
                    xprev = xt[mi][:msz, t0:t0 + tsz] if mi < 2 else xt[2][:44, t0:t0 + tsz]
                    dd = mpool.tile([128, 512], bf, tag="hwd")
                    nc.vector.tensor_tensor(dd[:msz, :tsz], hh[:msz, :tsz], xprev, op=ALU.subtract)
                    nc.vector.tensor_tensor(dd[:msz, :tsz], dd[:msz, :tsz], tt[:msz, :tsz], op=ALU.mult)
                    dst = xo[mi][:msz, t0:t0 + tsz] if mi < 2 else xo[2][:44, t0:t0 + tsz]
                    nc.vector.tensor_tensor(dst, dd[:msz, :tsz], xprev, op=ALU.add)

                for mi, (m0, msz) in enumerate(mcs300):
                    for tk in range(5):
                        t0, tsz = tk * 512, min(512, 2304 - tk * 512)
                        ph = psum.tile([128, 512], f32, tag="bulk")
                        pt = psum.tile([128, 512], f32, tag="bulk")
                        for kc in range(3):
                            nc.tensor.matmul(ph[:msz, :tsz], hw_sb[(L, 0)][:, kc, m0:m0 + msz],
                                             xt[kc][:, t0:t0 + tsz], start=(kc == 0), stop=(kc == 2))
                        for kc in range(3):
                            nc.tensor.matmul(pt[:msz, :tsz], hw_sb[(L, 1)][:, kc, m0:m0 + msz],
                                             xt[kc][:, t0:t0 + tsz], start=(kc == 0), stop=(kc == 2))
                        hw_epi(ph, pt, mi, m0, msz, t0, tsz)
                xt = xo

            # ---------------- inproj helper ----------------
            def inproj(nm, get_wt, nkc, rhs_fn, ntok, xproj_dst):
                """get_wt(d, mi, tk) -> lhsT AP [128, nkc, 128].
                Writes xproj_dst [(mi*2+d)*128+p, ntok] bf16."""
                ntc = (ntok + 511) // 512
                xpv = xproj_dst.rearrange("(mi d p) n -> d p mi n", d=2, p=128)
                for tk in range(ntc):
                    t0 = tk * 512
                    tsz = min(512, ntok - t0)
                    rhs_list = [rhs_fn(kc, t0, tsz) for kc in range(nkc)]
                    for d in range(2):
                        for half in range(2):
                            ob = mpool.tile([128, 4 * 512], bf, tag="ipo")
                            obv = ob.rearrange("p (mi n) -> p mi n", mi=4)
                            for mj in range(4):
                                mi = half * 4 + mj
                                w = get_wt(d, mi, tk)
                                ps = psum.tile([128, 512], f32, tag="bulk")
                                for kc in range(nkc):
                                    nc.tensor.matmul(ps[:, :tsz], w[:, kc, :],
                                                     rhs_list[kc], start=(kc == 0),
                                                     stop=(kc == nkc - 1))
                                if mi % 2 == 0:
                                    nc.scalar.activation(obv[:, mj, :tsz], ps[:, :tsz],
                                                         AF.Identity,
                                                         bias=bias_ap(nm, d, mi))
                                else:
                                    nc.vector.tensor_scalar(
                                        obv[:, mj, :tsz], ps[:, :tsz],
                                        bias_ap(nm, d, mi), None, op0=ALU.add)
                            nc.sync.dma_start(
                                xpv[d, :, half * 4:half * 4 + 4, t0:t0 + tsz],
                                obv[:, :, :tsz])

            def res_wt(nm):
                return lambda d, mi, tk: wres[(nm, d)][:, mi, :, :]

            def prep_whh(layer_idx):
                # emitted ahead of the preceding inproj so the DMA overlaps it
                for d in range(2):
                    nc.sync.dma_start(whh_sb[d][:], whh_pack[layer_idx, d])

            # ctx inproj (bias row zeroed in blocks; added via ones-row)
            prep_whh(0)
            inproj("ctx", res_wt("ctx"),
                   3, lambda kc, t0, tsz: xt[kc][:, t0:t0 + tsz],
                   NQ, xprojq_d)
            inproj("ctx", res_wt("ctx"),
                   3, lambda kc, t0, tsz: xt[kc][:, 256 + t0:256 + t0 + tsz],
                   NCtok, xprojc_d[0])

            # ---------------- recurrence ----------------
            def load_win(xp, w, nwin):
                wins = []
                for d in range(2):
                    wt = rpool.tile([128, 8 * W_WIN * 4], bf, tag=f"win{d}")
                    src_w = w if d == 0 else nwin - 1 - w
                    nc.sync.dma_start(
                        wt.rearrange("p (a x) -> p a x", x=W_WIN * 4),
                        xp[d, :, :, src_w * W_WIN * 4:(src_w + 1) * W_WIN * 4])
                    wins.append(wt.rearrange("p (a tt b) -> p a tt b", tt=W_WIN, b=4))
                return wins

            def bilstm(layer_idx, xproj_dram, Tlen, hseq):
                nc.vector.memset(x_uc[:], 0.0)
                hv = hview(hseq)
                hcv = [h_cur[j].rearrange("p (hc d b) -> p hc d b", d=2, b=4)
                       for j in range(2)]
                hiv = h_init.rearrange("p (hc d b) -> p hc d b", d=2, b=4)
                xp = xproj_dram.rearrange("(nc d p) n -> d p nc n", d=2, p=128)
                nwin = Tlen // W_WIN
                wins = load_win(xp, 0, nwin)
                for w in range(nwin):
                    wins_next = None
                    for ti in range(W_WIN):
                        if ti == 2 and w + 1 < nwin:
                            wins_next = load_win(xp, w + 1, nwin)
                        s = w * W_WIN + ti
                        sf, sb = s, Tlen - 1 - s
                        ps = psg.tile([128, 64], f32, tag="g")
                        pv = ps.rearrange("p (nc d b) -> p nc d b", d=2, b=4)
                        # xproj folded in via identity matmuls (no h dep; PE
                        # chews these while waiting on h). Exactly ONE
                        # start=True per step: it marks the whole 2KB psum
                        # zero-region pending-zero; every later first-touch
                        # write lands on zeroed cells, accumulation after.
                        first = [True]
                        for d in range(2):
                            tt = ti if d == 0 else W_WIN - 1 - ti
                            for nch in range(8):
                                nc.tensor.matmul(pv[:, nch, d, :], ident[:],
                                                 wins[d][:, nch, tt, :],
                                                 start=first[0], stop=False,
                                                 skip_group_check=True)
                                first[0] = False
                        cur = hcv[s % 2]
                        prev = hiv if s == 0 else hcv[(s + 1) % 2]
                        for d in range(2):
                            for nch in range(8):
                                for hc in range(2):
                                    nc.tensor.matmul(
                                        pv[:, nch, d, :],
                                        whh_sb[d][:, (hc * 8 + nch) * 128:(hc * 8 + nch) * 128 + 128],
                                        prev[:, hc, d, :], start=False,
                                        stop=(d == 1 and nch == 7 and hc == 1),
                                        skip_group_check=True)
                        # A = sigmoid([i, f, o, 2g]); f32: u = 2A_g - 1 would
                        # lose ~8 bits to cancellation in bf16
                        a_sb = spool.tile([128, 64], f32, tag="act")
                        nc.scalar.activation(a_sb[:], ps[:], AF.Sigmoid)
                        # u = tanh(g) = 2*A_g - 1 -> x_uc[:, 0:16]
                        nc.vector.tensor_scalar(x_uc[:, 0:16], a_sb[:, 48:64],
                                                2.0, -1.0, op0=ALU.mult, op1=ALU.add)
                        # P = [i, f] * [u, c]
                        pm = spool.tile([128, 32], f32, tag="pm")
                        nc.vector.tensor_tensor(pm[:], a_sb[:, 0:32], x_uc[:], op=ALU.mult)
                        # c_new = i*u + f*c -> x_uc[:, 16:32]
                        nc.vector.tensor_tensor(x_uc[:, 16:32], pm[:, 0:16],
                                                pm[:, 16:32], op=ALU.add)
                        tct = spool.tile([128, 16], bf, tag="tc")
                        nc.scalar.activation(tct[:], x_uc[:, 16:32], AF.Tanh)
                        # h = o * tanh(c) -> h_cur (feeds next step's matmuls)
                        nc.vector.tensor_tensor(h_cur[s % 2][:], a_sb[:, 32:48],
                                                tct[:], op=ALU.mult)
                        # persist h into hseq off the critical path (gpsimd)
                        nc.gpsimd.tensor_copy(hv[:, sf, :, 0, :], cur[:, :, 0, :])
                        nc.gpsimd.tensor_copy(hv[:, sb, :, 1, :], cur[:, :, 1, :])
                    wins = wins_next

            if KPH >= 2:
                bilstm(0, xprojq_d, J, hseq_q)
                bilstm(0, xprojc_d[0], T, hseq_c)

            if KPH >= 3:
                # ---------------- attention ----------------
                hq = hview(hseq_q)
                hc_v = hview(hseq_c)
                # w1.Hc -> w1hc_sb [1, 2048] bf16
                w1hc_sb = spool.tile([1, 2048], bf, tag="w1hc", bufs=1)
                for tk in range(4):
                    pw = psum.tile([1, 512], f32, tag="small", bufs=1)
                    for cch in range(4):
                        hcc, dd = cch // 2, cch % 2
                        nc.tensor.matmul(pw[:1, :],
                                         pvec["w1"][:, cch:cch + 1],
                                         hc_v[:, tk * 128:(tk + 1) * 128, hcc, dd, :],
                                         start=(cch == 0), stop=(cch == 3))
                    nc.scalar.activation(w1hc_sb[:1, tk * 512:(tk + 1) * 512], pw[:1, :], AF.Copy)
                # per-b attention
                w3u = {}
                uch = {}
                for b in range(4):
                    for cch in range(4):
                        hcc, dd = cch // 2, cch % 2
                        ut_ap = hq[:, :, hcc, dd, b]  # [128, 64]
                        t1 = spool.tile([128, 64], bf, tag="w3u", bufs=17)
                        nc.vector.tensor_scalar(t1[:], ut_ap, w3_sb[:, cch:cch + 1], None, op0=ALU.mult)
                        w3u[(b, cch)] = t1
                        pt = psum.tile([64, 128], bf, tag="tp")
                        nc.tensor.transpose(pt[:], ut_ap, ident[:])
                        t2 = spool.tile([64, 128], bf, tag="uch", bufs=17)
                        nc.vector.tensor_copy(t2[:], pt[:])
                        uch[(b, cch)] = t2
                w2u_sb = spool.tile([1, 256], bf, tag="w2u")
                for b in range(4):
                    pw = psum.tile([1, 64], f32, tag="small", bufs=1)
                    for cch in range(4):
                        hcc, dd = cch // 2, cch % 2
                        nc.tensor.matmul(pw[:1, :64],
                                         pvec["w2"][:, cch:cch + 1],
                                         hq[:, :, hcc, dd, b], start=(cch == 0), stop=(cch == 3))
                    nc.vector.tensor_scalar(w2u_sb[:1, b * 64:(b + 1) * 64], pw[:1, :64],
                                            attb_sb[:1, :1], None, op0=ALU.add)
                # S, softmax, Pn^T, expm
                pnT = {}
                expm_sb = [spool.tile([128, 4], bf, tag=f"expm{b}", name=f"expm{b}") for b in range(4)]
                for b in range(4):
                    for mc in range(4):
                        psS = psum.tile([128, 64], f32, tag="g")
                        for cch in range(4):
                            hcc, dd = cch // 2, cch % 2
                            nc.tensor.matmul(psS[:, :], hc_v[:, mc * 128:(mc + 1) * 128, hcc, dd, b],
                                             w3u[(b, cch)][:], start=(cch == 0), stop=False)
                        w1slice = w1hc_sb.rearrange("o (t b) -> o t b", b=4)[:1, mc * 128:(mc + 1) * 128, b]
                        nc.tensor.matmul(psS[:, :], w1slice, ones_row[:1, 0:64], start=False, stop=False)
                        nc.tensor.matmul(psS[:, :], ones_row[:1, 0:128],
                                         w2u_sb[:1, b * 64:(b + 1) * 64], start=False, stop=True)
                        mmax = spool.tile([128, 1], f32, tag="mx")
                        nc.vector.tensor_reduce(mmax[:], psS[:], axis=AX.X, op=ALU.max)
                        nc.scalar.activation(expm_sb[b][:, mc:mc + 1], mmax[:], AF.Exp)
                        eS = spool.tile([128, 64], bf, tag="eS")
                        nc.scalar.activation(eS[:], psS[:], AF.Exp)
                        rs = spool.tile([128, 1], f32, tag="rs")
                        nc.vector.tensor_reduce(rs[:], eS[:], axis=AX.X, op=ALU.add)
                        rr = spool.tile([128, 1], f32, tag="rr")
                        nc.vector.reciprocal(rr[:], rs[:])
                        pn = spool.tile([128, 64], bf, tag="pn")
                        nc.vector.tensor_scalar(pn[:], eS[:], rr[:], None, op0=ALU.mult)
                        ptp = psum.tile([64, 128], bf, tag="tp")
                        nc.tensor.transpose(ptp[:], pn[:], ident[:])
                        t3 = spool.tile([64, 128], bf, tag="pnT", bufs=17)
                        nc.vector.tensor_copy(t3[:], ptp[:])
                        pnT[(b, mc)] = t3
                # q2c attention weights over t
                q2cs = {}
                qrow_dram = dram.tile([4, 128], bf, tag="qrowd")
                for b in range(4):
                    zb = psum.tile([1, 4], f32, tag="small", bufs=1)
                    nc.tensor.matmul(zb[:1, :], ones_col[:, :1], expm_sb[b][:], start=True, stop=True)
                    z1 = spool.tile([1, 1], f32, tag="z1")
                    nc.vector.tensor_reduce(z1[:], zb[:1, :], axis=AX.X, op=ALU.add)
                    rz1 = spool.tile([1, 1], f32, tag="rz1")
                    nc.vector.reciprocal(rz1[:], z1[:])
                    rz1b = spool.tile([1, 1], bf, tag="rz1b")
                    nc.vector.tensor_copy(rz1b[:], rz1[:])
                    pzb = psum.tile([128, 1], f32, tag="tp")
                    nc.tensor.matmul(pzb[:, :1], ones_row[:1, 0:128], rz1b[:1, :1], start=True, stop=True)
                    rz = spool.tile([128, 1], f32, tag="rz")
                    nc.vector.tensor_copy(rz[:], pzb[:, :1])
                    # qattn row [1, 512] via DRAM bounce (partition -> free)
                    pq = psum.tile([4, 128], bf, tag="tp")
                    nc.tensor.transpose(pq[:4, :], expm_sb[b][:], ident[:])
                    qr4 = spool.tile([4, 128], bf, tag="qr4")
                    nc.vector.tensor_copy(qr4[:], pq[:4, :])
                    nc.sync.dma_start(qrow_dram[:], qr4[:])
                    qrow = spool.tile([1, 512], bf, tag="qrow")
                    nc.sync.dma_start(qrow[:1, :], qrow_dram.rearrange("a x -> (a x)")[None, :])
                    qbc = psum.tile([128, 512], f32, tag="bulk")
                    nc.tensor.matmul(qbc[:, :], ones_row[:1, 0:128], qrow[:1, :],
                                     start=True, stop=True)
                    for cch in range(4):
                        hcc, dd = cch // 2, cch % 2
                        tmp = mpool.tile([128, 512], bf, tag="qt")
                        nc.vector.tensor_tensor(tmp[:], hc_v[:, :, hcc, dd, b],
                                                qbc[:, :], op=ALU.mult)
                        qs = spool.tile([128, 1], f32, tag="qs")
                        nc.vector.tensor_reduce(qs[:], tmp[:], axis=AX.X, op=ALU.add)
                        qsc = spool.tile([128, 1], f32, tag="qsc", bufs=17)
                        nc.vector.tensor_scalar(qsc[:], qs[:], rz[:], None, op0=ALU.mult)
                        q2cs[(b, cch)] = qsc
                # c2qT per (b, fc): psum [128, 512]
                gt_c2q = [xpool.tile([128, 2304], bf, tag=("xo0" if fc == 3 else f"xt{fc}"), name=f"gtc{fc}") for fc in range(4)]
                for fc in range(4):
                    for b in range(4):
                        pc = psum.tile([128, 512], f32, tag="bulk")
                        for mc in range(4):
                            nc.tensor.matmul(pc[:, mc * 128:(mc + 1) * 128], uch[(b, fc)][:],
                                             pnT[(b, mc)][:], start=True, stop=True)
                        gv = gt_c2q[fc][:, :2048].rearrange("p (t b) -> p t b", b=4)
                        nc.scalar.activation(gv[:, :, b], pc[:], AF.Copy)
                # write GT chunks to DRAM
                for cch in range(4):
                    hcc, dd = cch // 2, cch % 2
                    g0 = xpool.tile([128, 2304], bf, tag="xo1")
                    gv0 = g0[:, :2048].rearrange("p (t b) -> p t b", b=4)
                    for b in range(4):
                        nc.vector.tensor_copy(gv0[:, :, b], hc_v[:, :, hcc, dd, b])
                    nc.sync.dma_start(GT_d[cch * 128:(cch + 1) * 128, :], g0[:, :2048])
                    nc.sync.dma_start(GT_d[(4 + cch) * 128:(5 + cch) * 128, :], gt_c2q[cch][:, :2048])
                    g2 = xpool.tile([128, 2304], bf, tag="xo2")
                    nc.vector.tensor_tensor(g2[:, :2048], g0[:, :2048], gt_c2q[cch][:, :2048], op=ALU.mult)
                    nc.sync.dma_start(GT_d[(8 + cch) * 128:(9 + cch) * 128, :], g2[:, :2048])
                    g3 = xpool.tile([128, 2304], bf, tag="xo1")
                    gv3 = g3[:, :2048].rearrange("p (t b) -> p t b", b=4)
                    for b in range(4):
                        nc.vector.tensor_scalar(gv3[:, :, b], hc_v[:, :, hcc, dd, b],
                                                q2cs[(b, cch)][:], None, op0=ALU.mult)
                    nc.sync.dma_start(GT_d[(12 + cch) * 128:(13 + cch) * 128, :], g3[:, :2048])

            if KPH >= 4:
                # ---------------- mod1 ----------------
                # per t-chunk: two batched GT loads of [128, 8, 512]
                gt_cache = {}
                gtd_v = GT_d.rearrange("(kc p) n -> p kc n", p=128)

                def gt_load(tk, tsz=512):
                    if tk not in gt_cache:
                        gt_cache.clear()
                        halves = []
                        for hf in range(2):
                            t = rpool.tile([128, 8 * 512], bf, tag=f"gtbig{hf}",
                                           bufs=1, name=f"gtbig{hf}_{tk}")
                            tv = t.rearrange("p (kc n) -> p kc n", kc=8)
                            nc.sync.dma_start(
                                tv[:, :, :tsz],
                                gtd_v[:, hf * 8:hf * 8 + 8,
                                      tk * 512:tk * 512 + tsz])
                            halves.append(tv)
                        gt_cache[tk] = halves

                    class GtView:
                        def __getitem__(self, idx):
                            _, kc, ns = idx
                            return gt_cache[tk][kc // 8][:, kc % 8, ns]
                    return GtView()

                prep_whh(1)
                inproj("mod1", stream_wt(mod1_wpk, 16), 16,
                       lambda kc, t0, tsz: gt_load(t0 // 512, tsz)[:, kc, :tsz],
                       NCtok, xprojc_d[1])
                bilstm(1, xprojc_d[1], T, hseq_m1)

                hm1 = hview(hseq_m1)

                def m1_rhs(kc, t0, tsz):
                    hcc, dd = kc // 2, kc % 2
                    return hm1[:, t0 // 4:(t0 + tsz) // 4, hcc, dd, :]

                prep_whh(2)
                inproj("mod2", stream_wt(mod2_wpk, 4), 4,
                       m1_rhs, NCtok, xprojc_d[2])
                bilstm(2, xprojc_d[2], T, hseq_m2)

                hm2 = hview(hseq_m2)

                def m2_rhs(kc, t0, tsz):
                    hcc, dd = kc // 2, kc % 2
                    return hm2[:, t0 // 4:(t0 + tsz) // 4, hcc, dd, :]

                prep_whh(3)
                inproj("dec", stream_wt(dec_wpk, 4), 4,
                       m2_rhs, NCtok, xprojc_d[3])
                bilstm(3, xprojc_d[3], T, hseq_dc)
                hdc = hview(hseq_dc)

            if KPH >= 5:
                # ---------------- p1 / p2 ----------------
                for tk in range(4):
                    t0 = tk * 512
                    gtv = gt_load(tk)
                    gts = [gtv[:, kc, :] for kc in range(16)]
                    for oi, (gw, mw, bw, hsv) in enumerate(
                            ((pvec["p1G"], pvec["p1M"], pvec["p1b"], hm2),
                             (pvec["p2G"], pvec["p2M"], pvec["p2b"], hdc))):
                        pp = psum.tile([1, 512], f32, tag="small", bufs=1)
                        for kc in range(16):
                            nc.tensor.matmul(pp[:1, :], gw[:, kc:kc + 1], gts[kc],
                                             start=(kc == 0), stop=False)
                        for kc in range(4):
                            hcc, dd = kc // 2, kc % 2
                            nc.tensor.matmul(pp[:1, :], mw[:, kc:kc + 1],
                                             hsv[:, tk * 128:(tk + 1) * 128, hcc, dd, :],
                                             start=False, stop=(kc == 3))
                        ostage = spool.tile([1, 512], f32, tag="ost", bufs=2)
                        nc.scalar.activation(ostage[:1, :], pp[:1, :], AF.Identity,
                                             bias=bw)
                        nc.sync.dma_start(out_d[oi:oi + 1, t0:t0 + 512], ostage[:1, :])

            if KPH < 5:
                zz = spool.tile([1, 2048], f32, tag='zz')
                nc.vector.memset(zz[:], 0.0)
                nc.sync.dma_start(out_d[0:1, :], zz[:1, :])
                nc.sync.dma_start(out_d[1:2, :], zz[:1, :])
    # post-pass: this walrus build allows only ONE sync wait per compute
    # instruction; split extra waits onto preceding same-engine NoOps.
    n_split = 0
    for bb in nc.m.functions[0].blocks:
        new = []
        for inst in bb.instructions:
            si = getattr(inst, 'sync_info', None)
            ow = list(si.on_wait) if si is not None and si.on_wait else []
            if len(ow) > 1:
                for w in ow[:-1]:
                    nop = mybir.InstNoOp(name=f"{inst.name}-ws{n_split}", ins=[], outs=[])
                    nop.engine = inst.engine
                    nop.sync_info = mybir.SyncInfo(on_wait=[w], on_update=[])
                    new.append(nop)
                    n_split += 1
                inst.sync_info = mybir.SyncInfo(on_wait=[ow[-1]],
                                                on_update=list(si.on_update or []))
            new.append(inst)
        bb.instructions[:] = new
    return nc


def kernel(**inputs):
    from concourse import bass_utils
    if "nc" not in _PROGRAM_CACHE:
        _PROGRAM_CACHE["nc"] = build_program()
    nc = _PROGRAM_CACHE["nc"]
    in_maps = [_build_host_inputs(inputs, core) for core in range(NC_)]
    res = bass_utils.run_bass_kernel_spmd(nc, in_maps, core_ids=list(range(NC_)))
    starts, ends = [], []
    for core in range(NC_):
        o = res.results[core]["out"]  # [2, 2048]
        starts.append(o[0].reshape(T, BL).T)
        ends.append(o[1].reshape(T, BL).T)
    start = np.concatenate(starts, axis=0).astype(np.float32)
    end = np.concatenate(ends, axis=0).astype(np.float32)
    return start, end

